# revision 16
# baseline (speedup 1.0000x reference)
"""Trainium2 Bass kernel for nn_Classifier_64587718197982 (spiking CNN).

Network (per reference):
  3x [conv3x3(C=128, pad=1, no bias) -> TDBN (batchnorm over T,B,H,W) -> LIF]
  -> mean over (H,W) -> mean over T -> FC(128->10)

Sharding: data-parallel over batch B=32 across 8 NeuronCores (4 images/core).
TDBN stats become a tiny [128,2] AllReduce per layer.

v2 design (fused pipeline, SBUF-resident):
  - y for all 32 images of the current layer lives in ONE SBUF f32 buffer
    [C, 32, HW] (128KB/partition); layer l+1's conv output overwrites slot
    (t,b) only after layer l's LIF consumed it (WAR tracked by Tile).
  - Phases: A = conv1 (all imgs); B = LIF1+conv2 interleaved per image;
    C = LIF2+conv3; D = LIF3+pool. TensorE never waits on LIF except at the
    3 BN-stats barriers.
  - conv: per image one [C,1024] PSUM tile (2 banks, halves bank-aligned);
    3x3 conv = 9 range-restricted shifted matmuls per half in fp16 (weights
    hi-rounded; spikes exact in fp16).
  - BN stats: Sum(y) rides the ScalarE PSUM->SBUF drain via accum_out;
    Sum(y^2) is one ScalarE Square (output to a PSUM scratch bank,
    accum_out -> slot). AllReduce of [Sum, SumSq]; then
    s' = 2*bn_w*rsqrt(var+eps), d' = 2*(bn_b - mean*s)  (r = mem/0.5 space,
    so the spike threshold is uniformly 1.0; x2 scaling is exact).
  - LIF (r-space, per image): z = s'*y + d' (ScalarE, per-partition
    scale/bias); gate g = 0.25*(1-S_prev) (GpSimd from fp16 spike);
    u *= g; u += z (VectorE); S = (u > 1) -> fp16 (VectorE). Spikes live in
    a 2-deep (t, t-1) ring per b-slot.
  - Phase D needs no spikes for a conv: gate+pool fuse into one VectorE op
    g = (u <= 1)*0.25 with accum_out counting non-spikes.
  - head: feat = 1 - negcount*4/8192; logits = fcw.T @ feat (one tiny
    matmul) + fcb via ScalarE bias drain; output [10, BL], transposed on
    host.
"""
import numpy as np
import ml_dtypes
from contextlib import ExitStack

import concourse.bass as bass
import concourse.mybir as mybir
import concourse.tile as tile
from concourse import bass_isa
from concourse import bacc
from concourse.bass_utils import run_bass_kernel_spmd

F32 = mybir.dt.float32
FP16 = mybir.dt.float16
AF = mybir.ActivationFunctionType
ALU = mybir.AluOpType

T, B, C, H, W = 8, 32, 128, 32, 32
NCORES = 8
BL = B // NCORES          # images per core per timestep
HW = H * W                # 1024
NIMG = T * BL             # 32 images per core
NHALF = 2                 # psum banks per image (16 rows x 32 cols = 512)
RH = H // NHALF           # rows per half
DECAY = 0.25
THRESH = 0.5
BN_EPS = 1e-5
POOL_N = float(T * HW)    # pooling divisor 8192

# fp16 term counts for L1 conv (1 = single fp16 term; 3 = ~fp32 via hi/lo)
TERMS_L1 = 1
STATS_IMGS = NIMG         # images contributing to BN stats (<32 = sampled)
D_FP16 = True             # phase-D membrane dtype fp16 (L3 only; no conv after)
U16 = False               # phases B/C membrane dtype fp16
DEBUG = False

# conv shift order: (1,1) first so the start=True matmul covers the full half
SHIFTS = [(1, 1)] + [(dy, dx) for dy in range(3) for dx in range(3)
                     if not (dy == 1 and dx == 1)]


def _conv_image(nc, psum_pool, ybuf, sum_sl, sq_sl, img, terms):
    """One image conv: terms = list of (w_sb[C,9,C], x3d[C,H,W]).
    Drains f32 y into ybuf[:, img] with Sum(y) accum; Sum(y^2) via Square."""
    pt = psum_pool.tile([C, HW], F32, tag="psum")
    p3 = pt.rearrange("c (r w) -> c r w", r=H)
    for h in range(NHALF):
        r_base = h * RH
        n_mm = len(SHIFTS) * len(terms)
        i_mm = 0
        for (dy, dx) in SHIFTS:
            oy, ox = dy - 1, dx - 1
            r0 = max(r_base, -oy)
            r1 = min(r_base + RH, H - oy)
            c0 = max(0, -ox)
            c1 = min(W, W - ox)
            k = dy * 3 + dx
            for (w_sb, x3d) in terms:
                nc.tensor.matmul(
                    p3[:, r0:r1, c0:c1],
                    w_sb[:, k, :],
                    x3d[:, r0 + oy:r1 + oy, c0 + ox:c1 + ox],
                    start=(i_mm == 0), stop=(i_mm == n_mm - 1),
                )
                i_mm += 1
    if img < STATS_IMGS:
        nc.scalar.activation(ybuf[:, img], pt, AF.Identity,
                             accum_out=sum_sl[:, img:img + 1])
        nc.scalar.activation(pt, pt, AF.Square,
                             accum_out=sq_sl[:, img:img + 1])
    else:
        nc.scalar.copy(out=ybuf[:, img], in_=pt)


def _layer_stats(nc, sb1, sum_sl, sq_sl, bnw, bnb, eps_t, cc_in, cc_out,
                 lname, dbg_cc=None, dbg_ccr=None):
    """Reduce per-image sums, AllReduce, compute s' = 2s and d' = 2d."""
    cc = sb1.tile([C, 2], F32, tag=f"cc{lname}")
    nc.vector.tensor_reduce(cc[:, 0:1], sum_sl,
                            axis=mybir.AxisListType.X, op=ALU.add)
    nc.vector.tensor_reduce(cc[:, 1:2], sq_sl,
                            axis=mybir.AxisListType.X, op=ALU.add)
    if dbg_cc is not None:
        nc.sync.dma_start(out=dbg_cc[:, :], in_=cc)
    nc.sync.dma_start(out=cc_in[:, :], in_=cc)
    nc.gpsimd.collective_compute(
        "AllReduce", ALU.add,
        replica_groups=[list(range(NCORES))],
        ins=[cc_in[:, :]], outs=[cc_out[:, :]],
    )
    ccr = sb1.tile([C, 2], F32, tag=f"ccr{lname}")
    nc.sync.dma_start(out=ccr, in_=cc_out[:, :])
    if dbg_ccr is not None:
        nc.sync.dma_start(out=dbg_ccr[:, :], in_=ccr)
    n_tot = float(STATS_IMGS * HW * NCORES)
    mean = sb1.tile([C, 1], F32, tag=f"mean{lname}")
    nc.vector.tensor_scalar(mean, ccr[:, 0:1], 1.0 / n_tot, None,
                            op0=ALU.mult)
    ex2 = sb1.tile([C, 1], F32, tag=f"ex2{lname}")
    nc.vector.tensor_scalar(ex2, ccr[:, 1:2], 1.0 / n_tot, None,
                            op0=ALU.mult)
    var = sb1.tile([C, 1], F32, tag=f"var{lname}")
    nc.vector.tensor_tensor(var, mean, mean, op=ALU.mult)
    nc.vector.tensor_tensor(var, ex2, var, op=ALU.subtract)
    sd = sb1.tile([C, 1], F32, tag=f"sd{lname}")
    nc.scalar.activation(sd, var, AF.Sqrt, bias=eps_t, scale=1.0)
    inv = sb1.tile([C, 1], F32, tag=f"inv{lname}")
    nc.vector.reciprocal(out=inv, in_=sd)
    s1 = sb1.tile([C, 1], F32, tag=f"s1{lname}")
    nc.vector.tensor_tensor(s1, bnw, inv, op=ALU.mult)
    # v-space: v = mem/s; v_t = g*v + y + d/s; spike: v > 0.5/s (s > 0)
    rs = sb1.tile([C, 1], F32, tag=f"rs{lname}")
    nc.vector.reciprocal(out=rs, in_=s1)
    thr = sb1.tile([C, 1], F32, tag=f"thr{lname}")
    nc.vector.tensor_scalar(thr, rs, THRESH, None, op0=ALU.mult)
    dp = sb1.tile([C, 1], F32, tag=f"dp{lname}")
    nc.vector.tensor_tensor(dp, bnb, rs, op=ALU.mult)
    nc.vector.tensor_tensor(dp, dp, mean, op=ALU.subtract)
    return dp, thr


def build():
    nc = bacc.Bacc("TRN2", target_bir_lowering=False, debug=False,
                   num_devices=NCORES)

    # --- I/O ---
    xhi_d = nc.dram_tensor("xhi", [T, BL, C, HW], FP16, kind="ExternalInput")
    xlo_d = None
    if TERMS_L1 >= 3:
        xlo_d = nc.dram_tensor("xlo", [T, BL, C, HW], FP16,
                               kind="ExternalInput")
    w_d = {}
    for l in (1, 2, 3):
        w_d[(l, "hi")] = nc.dram_tensor(f"w{l}hi", [C, 9, C], FP16,
                                        kind="ExternalInput")
    if TERMS_L1 >= 2:
        w_d[(1, "lo")] = nc.dram_tensor("w1lo", [C, 9, C], FP16,
                                        kind="ExternalInput")
    bn_d = {}
    for l in (1, 2, 3):
        bn_d[(l, "w")] = nc.dram_tensor(f"bnw{l}", [C, 1], F32,
                                        kind="ExternalInput")
        bn_d[(l, "b")] = nc.dram_tensor(f"bnb{l}", [C, 1], F32,
                                        kind="ExternalInput")
    fcw_d = nc.dram_tensor("fcw", [C, 10], F32, kind="ExternalInput")
    fcb_d = nc.dram_tensor("fcb", [10, 1], F32, kind="ExternalInput")
    out_d = nc.dram_tensor("out", [10, BL], F32, kind="ExternalOutput")
    dbg = {}
    if DEBUG:
        for nm, shp in (("y1", [C, HW]), ("cc1", [C, 2]), ("ccr1", [C, 2]),
                        ("sp1", [C, 1]), ("dp1", [C, 1]), ("spk1", [C, HW]),
                        ("y2", [C, HW]), ("sp2", [C, 1]), ("dp2", [C, 1]),
                        ("spk2", [C, HW]), ("y3", [C, HW]),
                        ("pool", [C, T * BL]), ("feat", [C, BL])):
            dt_ = FP16 if nm.startswith("spk") else F32
            dbg[nm] = nc.dram_tensor(f"dbg_{nm}", shp, dt_,
                                     kind="ExternalOutput")

    # --- internal DRAM for collectives ---
    cc_bufs = {}
    for l in (1, 2, 3):
        cc_bufs[l] = (
            nc.dram_tensor(f"cc_in{l}", [C, 2], F32),
            nc.dram_tensor(f"cc_out{l}", [C, 2], F32, addr_space="Shared"),
        )

    with ExitStack() as ctx:
        tc = ctx.enter_context(tile.TileContext(nc))
        sb1 = ctx.enter_context(tc.tile_pool(name="sb1", bufs=1))
        xpool = ctx.enter_context(tc.tile_pool(name="xpool", bufs=2))
        zpool = ctx.enter_context(tc.tile_pool(name="zpool", bufs=2))
        gpool = ctx.enter_context(tc.tile_pool(name="gpool", bufs=2))
        psum_pool = ctx.enter_context(
            tc.tile_pool(name="psum", bufs=3, space="PSUM"))
        fcp_pool = ctx.enter_context(
            tc.tile_pool(name="fcp", bufs=1, space="PSUM"))

        # --- load constants ---
        w_sb = {}
        for key, dt_ in w_d.items():
            w_sb[key] = sb1.tile([C, 9, C], FP16, name=f"w{key[0]}{key[1]}",
                                 tag=f"w{key[0]}{key[1]}")
            nc.sync.dma_start(out=w_sb[key], in_=dt_[:, :, :])
        bn_sb = {}
        for key, dt_ in bn_d.items():
            bn_sb[key] = sb1.tile([C, 1], F32, name=f"bn{key[1]}{key[0]}",
                                  tag=f"bn{key[1]}{key[0]}")
            nc.sync.dma_start(out=bn_sb[key], in_=dt_[:, :])
        fcw_sb = sb1.tile([C, 10], F32, tag="fcw")
        nc.sync.dma_start(out=fcw_sb, in_=fcw_d[:, :])
        fcb_sb = sb1.tile([10, 1], F32, tag="fcb")
        nc.sync.dma_start(out=fcb_sb, in_=fcb_d[:, :])
        eps_t = sb1.tile([C, 1], F32, tag="eps")
        nc.vector.memset(eps_t, BN_EPS)

        # --- persistent state ---
        ybuf = sb1.tile([C, NIMG, HW], F32, tag="ybuf")      # 128KB/part
        umem = sb1.tile([C, BL, HW], FP16 if U16 else F32,
                        tag="umem")        # 16KB/part
        if D_FP16:
            udm = sb1.tile([C, BL, HW], FP16, tag="udm")     # 8KB/part
        else:
            udm = umem
        ring = sb1.tile([C, 2, BL, HW], FP16, tag="ring")    # 16KB/part
        ring4 = ring.rearrange("c s b (h w) -> c s b h w", h=H)
        sum_sl = {}
        sq_sl = {}
        for l in (1, 2, 3):
            sum_sl[l] = sb1.tile([C, STATS_IMGS], F32, name=f"sum{l}",
                                 tag=f"sum{l}")
            sq_sl[l] = sb1.tile([C, STATS_IMGS], F32, name=f"sq{l}",
                                tag=f"sq{l}")
        poolneg = sb1.tile([C, T, BL], F32, tag="poolneg")

        # =============== phase A: conv L1 ===============
        for t in range(T):
            for b in range(BL):
                xhi = xpool.tile([C, HW], FP16, tag="xhi")
                nc.sync.dma_start(out=xhi, in_=xhi_d[t, b, :, :])
                xhi3 = xhi.rearrange("c (h w) -> c h w", h=H)
                terms = [(w_sb[(1, "hi")], xhi3)]
                if TERMS_L1 >= 2:
                    terms.append((w_sb[(1, "lo")], xhi3))
                if TERMS_L1 >= 3:
                    xlo = xpool.tile([C, HW], FP16, tag="xlo")
                    nc.sync.dma_start(out=xlo, in_=xlo_d[t, b, :, :])
                    terms.append((w_sb[(1, "hi")],
                                  xlo.rearrange("c (h w) -> c h w", h=H)))
                _conv_image(nc, psum_pool, ybuf,
                            sum_sl[1], sq_sl[1], t * BL + b, terms)

        dp1, thr1 = _layer_stats(nc, sb1, sum_sl[1], sq_sl[1],
                                bn_sb[(1, "w")], bn_sb[(1, "b")], eps_t,
                                *cc_bufs[1], "l1",
                                dbg_cc=dbg.get("cc1"),
                                dbg_ccr=dbg.get("ccr1"))
        if DEBUG:
            nc.sync.dma_start(out=dbg["y1"][:, :], in_=ybuf[:, 0])
            nc.sync.dma_start(out=dbg["sp1"][:, :], in_=thr1)
            nc.sync.dma_start(out=dbg["dp1"][:, :], in_=dp1)

        # =============== fused LIF + next-layer conv ===============
        def lif_conv_phase(dp, thr, wkey, dbg_spk=None):
            """v-space LIF for the layer whose y is in ybuf, spikes -> ring,
            and immediately conv the spike image into ybuf[:, img]."""
            for t in range(T):
                for b in range(BL):
                    img = t * BL + b
                    if t == 0:
                        nc.vector.tensor_scalar(umem[:, b], ybuf[:, img],
                                                dp, None, op0=ALU.add)
                    else:
                        g = gpool.tile([C, HW], FP16, tag="g")
                        nc.gpsimd.tensor_scalar(g, ring[:, (t - 1) % 2, b],
                                                -DECAY, DECAY,
                                                op0=ALU.mult, op1=ALU.add)
                        nc.vector.tensor_tensor(umem[:, b], umem[:, b], g,
                                                op=ALU.mult)
                        nc.vector.scalar_tensor_tensor(
                            umem[:, b], ybuf[:, img], dp, umem[:, b],
                            op0=ALU.add, op1=ALU.add)
                    S = ring[:, t % 2, b]
                    nc.vector.tensor_scalar(S, umem[:, b], thr, None,
                                            op0=ALU.is_gt)
                    if dbg_spk is not None and t == 0 and b == 0:
                        nc.sync.dma_start(out=dbg_spk[:, :], in_=S)
                    _conv_image(nc, psum_pool, ybuf,
                                sum_sl[wkey], sq_sl[wkey], img,
                                [(w_sb[(wkey, "hi")], ring4[:, t % 2, b])])

        # phase B: LIF1 + conv2
        lif_conv_phase(dp1, thr1, 2,
                       dbg_spk=dbg.get("spk1"))
        dp2, thr2 = _layer_stats(nc, sb1, sum_sl[2], sq_sl[2],
                                bn_sb[(2, "w")], bn_sb[(2, "b")], eps_t,
                                *cc_bufs[2], "l2")
        if DEBUG:
            nc.sync.dma_start(out=dbg["y2"][:, :], in_=ybuf[:, 0])
            nc.sync.dma_start(out=dbg["sp2"][:, :], in_=thr2)
            nc.sync.dma_start(out=dbg["dp2"][:, :], in_=dp2)
        # phase C: LIF2 + conv3
        lif_conv_phase(dp2, thr2, 3,
                       dbg_spk=dbg.get("spk2"))
        dp3, thr3 = _layer_stats(nc, sb1, sum_sl[3], sq_sl[3],
                                bn_sb[(3, "w")], bn_sb[(3, "b")], eps_t,
                                *cc_bufs[3], "l3")
        if DEBUG:
            nc.sync.dma_start(out=dbg["y3"][:, :], in_=ybuf[:, 0])

        # =============== phase D: LIF3 + pooled non-spike counts ==========
        for t in range(T):
            for b in range(BL):
                img = t * BL + b
                if t == 0:
                    nc.vector.tensor_scalar(udm[:, b], ybuf[:, img],
                                            dp3, None, op0=ALU.add)
                else:
                    nc.vector.tensor_tensor(udm[:, b], udm[:, b],
                                            ring[:, (t - 1) % 2, b],
                                            op=ALU.mult)
                    nc.vector.scalar_tensor_tensor(
                        udm[:, b], ybuf[:, img], dp3, udm[:, b],
                        op0=ALU.add, op1=ALU.add)
                # sg = sign(thr - v): +1 no-spike, -1 spike; accum = #non-#spk
                sg = ring[:, t % 2, b]
                nc.scalar.activation(sg, udm[:, b], AF.Sign,
                                     bias=thr3, scale=-1.0,
                                     accum_out=poolneg[:, t, b:b + 1])
                # in-place: gate = 0.125*sg + 0.125 in {0.25, 0}
                nc.gpsimd.tensor_scalar(sg, sg, 0.125, 0.125,
                                        op0=ALU.mult, op1=ALU.add)

        if DEBUG:
            nc.sync.dma_start(
                out=dbg["pool"][:, :],
                in_=poolneg.rearrange("c t b -> c (t b)"))
        # =============== head: pooling + FC ===============
        negfeat = sb1.tile([C, BL], F32, tag="negfeat")
        for b in range(BL):
            nc.vector.tensor_reduce(negfeat[:, b:b + 1], poolneg[:, :, b],
                                    axis=mybir.AxisListType.X, op=ALU.add)
        # accum counted sign(1-u): q = #non - #spk; spike_frac = 0.5 - q/16384
        feat = sb1.tile([C, BL], F32, tag="feat")
        nc.vector.tensor_scalar(feat, negfeat, -1.0 / (2 * POOL_N), 0.5,
                                op0=ALU.mult, op1=ALU.add)
        if DEBUG:
            nc.sync.dma_start(out=dbg["feat"][:, :], in_=feat)
        pfc = fcp_pool.tile([10, BL], F32, tag="pfc")
        nc.tensor.matmul(pfc, fcw_sb, feat, start=True, stop=True)
        ofin = sb1.tile([10, BL], F32, tag="ofin")
        nc.scalar.activation(ofin, pfc, AF.Identity, bias=fcb_sb, scale=1.0)
        nc.sync.dma_start(out=out_d[:, :], in_=ofin)

    nc.compile()
    return nc


_NC_CACHE = {}


def _get_nc():
    if "nc" not in _NC_CACHE:
        _NC_CACHE["nc"] = build()
    return _NC_CACHE["nc"]


def _hi_lo(a):
    hi = a.astype(np.float16)
    lo = (a - hi.astype(np.float32)).astype(np.float16)
    return hi, lo


def make_in_maps(inp, conv_ws, bns, fc_w, fc_b):
    """Build the 8 per-core input maps from full (numpy) model inputs."""
    common = {}
    for li, w in enumerate(conv_ws, start=1):
        wt = np.ascontiguousarray(
            w.transpose(1, 2, 3, 0).reshape(C, 9, C))  # [I, k, O]
        hi, lo = _hi_lo(wt)
        common[f"w{li}hi"] = hi
        if li == 1 and TERMS_L1 >= 2:
            common["w1lo"] = lo
        common[f"bnw{li}"] = np.ascontiguousarray(
            bns[li - 1][0].reshape(C, 1))
        common[f"bnb{li}"] = np.ascontiguousarray(
            bns[li - 1][1].reshape(C, 1))
    common["fcw"] = np.ascontiguousarray(fc_w.T)          # [C, 10]
    common["fcb"] = np.ascontiguousarray(fc_b.reshape(10, 1))

    in_maps = []
    for cid in range(NCORES):
        xc = np.ascontiguousarray(
            inp[:, cid * BL:(cid + 1) * BL].reshape(T, BL, C, HW))
        xhi, xlo = _hi_lo(xc)
        m = dict(common)
        m["xhi"] = xhi
        if TERMS_L1 >= 3:
            m["xlo"] = xlo
        in_maps.append(m)
    return in_maps


def kernel(inp, conv_w1, conv_w2, conv_w3, bn_w1, bn_b1, bn_w2, bn_b2,
           bn_w3, bn_b3, fc_w, fc_b):
    inp = np.asarray(inp, dtype=np.float32)
    ws = [np.asarray(w, dtype=np.float32) for w in (conv_w1, conv_w2, conv_w3)]
    bns = [(np.asarray(bn_w1, np.float32), np.asarray(bn_b1, np.float32)),
           (np.asarray(bn_w2, np.float32), np.asarray(bn_b2, np.float32)),
           (np.asarray(bn_w3, np.float32), np.asarray(bn_b3, np.float32))]
    fc_w = np.asarray(fc_w, np.float32)
    fc_b = np.asarray(fc_b, np.float32)

    nc = _get_nc()
    in_maps = make_in_maps(inp, ws, bns, fc_w, fc_b)
    res = run_bass_kernel_spmd(nc, in_maps, core_ids=list(range(NCORES)))
    out = np.concatenate(
        [r["out"].reshape(10, BL).T for r in res.results], axis=0)
    return out.astype(np.float32)
